# revision 33
# baseline (speedup 1.0000x reference)
"""CNN-LSTM Trainium2 kernel: 8-way tensor-parallel over the 4H gate dim.

Strategy (v2):
- Host folds the hidden projection into the gate weights (M00 = W_hh0 @ W_hr0,
  M10 = W_ih1 @ W_hr0, M11 = W_hh1 @ W_hr1) so the recurrence runs entirely on
  the sharded s = sigmoid(o)*tanh(c) vectors (H=1024, 128 per core).
- The two layers' gate GEMMs run CONCURRENTLY in the two halves of the PE
  array via column tiling: gates0 PSUM sits at partitions 0:64 (col-group
  0-1), gates1 at 64:128 (col-group 2-3); interleaved matmul pairs execute
  with ~4ns stagger.  All L1 elementwise state lives on partitions 64:128.
- Gate row order is [i, f, o, g] so each layer needs one sigmoid op over
  [*, 0:384] and one tanh over [*, 384:512].
- The s-exchange is SPLIT: X0(n) broadcasts s0T[n] (16KB) as soon as the L0
  chain produces it mid-superstep; X1(n) broadcasts s1T[n-2] later.  Each has
  its own rsem set (rs0/rs1 x3 buffers) and fires via its own trigger, giving
  every consumer nearly a full superstep of flight slack and taking the L1
  chain off the critical cycle.
- te stream order [.. D/E(n) pairs, T0(n), T1(n), A/C(n+1) pairs ..] keeps PE
  idle gaps under the ~3.4us HAM re-throttle window.
- Epilogue: h1 = P1 @ s1 + softmax, sharded over T (16 steps/core).
"""
import sys
import os
import numpy as np

sys.path.insert(0, "/opt/trn_rl_repo")

import concourse.bass as bass  # noqa: E402
import concourse.bacc as bacc  # noqa: E402
import concourse.mybir as mybir  # noqa: E402
from concourse.bass_utils import run_bass_kernel_spmd  # noqa: E402
import ml_dtypes  # noqa: E402

BF = mybir.dt.bfloat16
F32 = mybir.dt.float32
AF = mybir.ActivationFunctionType

B, T, E, H, V = 64, 128, 512, 1024, 10000
NCORES = 8
TRACE = False
LAST_EXEC_NS = None
_CACHE = {}


def _install_trace_hook():
    import types, contextlib, ctypes

    if "antenv.axon_hooks" in sys.modules:
        return
    mod = types.ModuleType("antenv.axon_hooks")
    mod._hook = None
    mod.set_axon_ntff_profile_hook = lambda h: setattr(mod, "_hook", h)
    mod.get_axon_ntff_profile_hook = lambda: mod._hook
    sys.modules["antenv.axon_hooks"] = mod
    import antenv

    antenv.axon_hooks = mod
    so_path = "/opt/axon/libaxon_pjrt.so"
    try:
        lib = ctypes.CDLL(so_path)
    except OSError:
        return
    if not hasattr(lib, "axon_start_nrt_profile"):
        return
    lib.axon_start_nrt_profile.argtypes = [ctypes.POINTER(ctypes.c_int64), ctypes.c_size_t]
    lib.axon_start_nrt_profile.restype = ctypes.c_int64
    lib.axon_stop_nrt_profile.argtypes = [ctypes.c_char_p]
    lib.axon_stop_nrt_profile.restype = ctypes.c_int64

    @contextlib.contextmanager
    def _hook(output_dir, device_ids):
        import jax

        jax.devices()
        if device_ids:
            ids = (ctypes.c_int64 * len(device_ids))(*device_ids)
            rc = lib.axon_start_nrt_profile(ids, len(device_ids))
        else:
            rc = lib.axon_start_nrt_profile(None, 0)
        if rc != 0:
            raise RuntimeError(f"axon_start_nrt_profile rc={rc}")
        try:
            yield
        finally:
            n = lib.axon_stop_nrt_profile(str(output_dir).encode())
            print(f"profile: {n} file(s) -> {output_dir}", file=sys.stderr)

    mod.set_axon_ntff_profile_hook(_hook)


def build(t_steps=T, dump=False):
    NS = t_steps + 3  # supersteps 0 .. t_steps+2
    TSH = t_steps // NCORES  # epilogue steps per core

    nc = bacc.Bacc("TRN2", target_bir_lowering=False, debug=False, num_devices=8,
                   num_swdge_queues=2)

    # ---- I/O ----
    w0d = nc.dram_tensor("w0", [13, 128, 512], BF, kind="ExternalInput")
    w1d = nc.dram_tensor("w1", [17, 128, 512], BF, kind="ExternalInput")
    p1d = nc.dram_tensor("p1w", [8, 128, 512], BF, kind="ExternalInput")
    xtd = nc.dram_tensor("xT", [512, t_steps * 64], BF, kind="ExternalInput")
    onesd = nc.dram_tensor("ones", [128, 64], BF, kind="ExternalInput")
    idend = nc.dram_tensor("iden", [128, 128], BF, kind="ExternalInput")
    rankd = nc.dram_tensor("rank", [1, 1], mybir.dt.int32, kind="ExternalInput")
    rank16d = nc.dram_tensor("rank16", [1, 1], mybir.dt.int32, kind="ExternalInput")
    yd = nc.dram_tensor("y", [64, TSH, 512], F32, kind="ExternalOutput")
    s1store = nc.dram_tensor(
        "s1store", [t_steps, 128 * 512], BF,
        kind="ExternalOutput" if dump else "Internal",
    )

    # ---- SBUF ----
    # L0 state lives on partitions 0:64, L1 state on 64:128 so the two
    # layers' GEMMs run concurrently in the two PE column halves.
    W0 = nc.alloc_sbuf_tensor("W0", [128, 13 * 512], BF)
    W1 = nc.alloc_sbuf_tensor("W1", [128, 17 * 512], BF)
    P1S = nc.alloc_sbuf_tensor("P1S", [128, 8 * 512], BF)
    Gb = [nc.alloc_sbuf_tensor(f"G{q}", [128, 1024], BF) for q in range(3)]
    SS = [nc.alloc_sbuf_tensor(f"SS{p}", [128, 128], BF) for p in range(2)]
    XT = nc.alloc_sbuf_tensor("XT", [128, 2 * 256], BF)
    ONES = nc.alloc_sbuf_tensor("ONES", [128, 64], BF)
    IDN = nc.alloc_sbuf_tensor("IDN", [128, 128], BF)
    actb = nc.alloc_sbuf_tensor("actb", [128, 512], F32)  # i,f,o,g; L0|L1 halves
    cbuf = nc.alloc_sbuf_tensor("cbuf", [128, 128], F32)  # c0 | c1 halves
    thc = nc.alloc_sbuf_tensor("thc", [128, 128], F32)
    sS = [nc.alloc_sbuf_tensor(f"sS{p}", [128, 128], BF) for p in range(2)]
    es1 = [nc.alloc_sbuf_tensor(f"es1_{p}", [128, 512], BF) for p in range(2)]
    emx = nc.alloc_sbuf_tensor("emx", [64, 8], F32)
    ebuf = nc.alloc_sbuf_tensor("ebuf", [64, 512], F32)

    # ---- PSUM (7 banks) ----
    ps_g0 = [nc.alloc_psum_tensor(f"psg0_{p}", [64, 512], F32) for p in range(2)]
    ps_g1b = [nc.alloc_psum_tensor(f"psg1_{p}", [128, 512], F32) for p in range(2)]
    ps_g1 = [t[64:128, :] for t in ps_g1b]  # gates1 at PE col-group 2-3
    ps_t = [nc.alloc_psum_tensor(f"pst_{p}", [128, 128], BF) for p in range(2)]
    ps_e = [nc.alloc_psum_tensor(f"pse_{p}", [64, 512], F32) for p in range(2)]

    # ---- semaphores ----
    rs = [nc.alloc_semaphore(f"rs_{q}") for q in range(3)]
    ls = nc.alloc_semaphore("ls")
    prep = nc.alloc_semaphore("prep")
    pe = nc.alloc_semaphore("pe")
    acts = nc.alloc_semaphore("acts")
    dve = nc.alloc_semaphore("dve")
    xdma = nc.alloc_semaphore("xdma")
    sdma = nc.alloc_semaphore("sdma")
    edma = nc.alloc_semaphore("edma")
    idma = nc.alloc_semaphore("idma")
    init = nc.alloc_semaphore("init")
    ydma = nc.alloc_semaphore("ydma")

    # ---- schedule booleans ----
    def flags(n):
        return dict(
            L0=(n <= t_steps - 1),          # gates0 / s0[n]
            L0dep=(1 <= n <= t_steps - 1),  # D-block
            L1=(2 <= n <= t_steps + 1),     # gates1 / s1[n-2]
            Edep=(3 <= n <= t_steps + 1),   # E-block
            X=(n <= t_steps + 1),           # pair exchange X(n)
        )

    # flat all-to-all pair broadcast: 8 senders x 2 rsem incs per
    # exchange.  X(n) carries [s0T[n] | s1T[n-2]].
    def rth(m):
        return 16 * (m // 3 + 1)

    def cnt_x(m):  # exchanges with index <= m
        return min(m, t_steps + 1) + 1 if m >= 0 else 0

    # ---- analytic milestone tables (cumulative then_inc counts) ----
    pe_g0, pe_g1, pe_t = {}, {}, {}
    ag0, ag1, at0, at1 = {}, {}, {}, {}
    dc0, dc1, ds0, ds1, dcp = {}, {}, {}, {}, {}
    pq, pf = {}, {}
    xd_cnt, st_cnt = {}, {}
    c_pe = c_a = c_d = c_pr = 0
    c_x = 1
    c_st = 0
    for n in range(NS):
        f = flags(n)
        if f["X"]:
            c_pr += 1
        pq[n] = c_pr
        if f["L0"]:
            c_pe += 1
        pe_g0[n] = c_pe
        if f["L1"]:
            c_pe += 1
        pe_g1[n] = c_pe
        if f["X"]:
            c_pe += 1
        pe_t[n] = c_pe
        if f["L0"]:
            c_a += 1
        ag0[n] = c_a
        if f["L1"]:
            c_a += 1
        ag1[n] = c_a
        if f["L0"]:
            c_a += 1
        at0[n] = c_a
        if f["L1"]:
            c_a += 1
        at1[n] = c_a
        if f["L0"]:
            c_d += 1
        dc0[n] = c_d
        if f["L1"]:
            c_d += 1
        dc1[n] = c_d
        if f["L0"]:
            c_d += 1
        ds0[n] = c_d
        if f["L1"]:
            c_d += 1
        ds1[n] = c_d
        if f["X"]:
            c_d += 1
        dcp[n] = c_d
        if n + 1 <= t_steps - 1:
            c_x += 1
        xd_cnt[n] = c_x
        if 3 <= n <= t_steps + 2:
            c_st += 1
        st_cnt[n] = c_st
    PTOT, ATOT, DTOT = c_pe, c_a, c_d

    with nc.Block() as block:

        # ================= GPSIMD =================
        @block.gpsimd
        def _(g):
            with g.register("rank") as rank, g.register("urow") as urow, \
                    g.register("r16") as r16:
                g.load(rank, rankd.ap())
                g.load(r16, rank16d.ap())
                g.dma_start(
                    out=W0.rearrange("p (k c) -> p k c", k=13),
                    in_=w0d.rearrange("k p c -> p k c"),
                ).then_inc(idma, 16)
                g.dma_start(
                    out=W1.rearrange("p (k c) -> p k c", k=17),
                    in_=w1d.rearrange("k p c -> p k c"),
                ).then_inc(idma, 16)
                g.dma_start(
                    out=P1S.rearrange("p (k c) -> p k c", k=8),
                    in_=p1d.rearrange("k p c -> p k c"),
                ).then_inc(idma, 16)
                g.dma_start(out=ONES[:, :], in_=onesd[:, :]).then_inc(idma, 16)
                g.dma_start(out=IDN[:, :], in_=idend[:, :]).then_inc(idma, 16)
                g.wait_ge(idma, 80)
                g.memset(cbuf[:, :], 0.0)
                g.memset(SS[0][:, :], 0.0)
                g.memset(SS[1][:, :], 0.0)
                g.memset(sS[0][:, :], 0.0)
                g.memset(sS[1][:, :], 0.0)
                g.memset(emx[:, :], 0.0).then_inc(init, 1)
                g.bir_kernel_barrier_wait([list(range(8))])
                rdests8 = [(0, k) for k in range(8)]
                for n in range(NS):
                    f = flags(n)
                    if f["X"]:
                        for r in range(8):
                            with g.If_eq(rank, r):
                                g.remote_dma_broadcast(
                                    out_ap=Gb[n % 3][:, r * 128:(r + 1) * 128],
                                    in_ap=SS[n % 2][:, :],
                                    remote_sem=rs[n % 3],
                                    local_sem=ls,
                                    rdests=rdests8,
                                ).then_inc(prep, 1)
                        # fire once the pair copy landed in SS; Gb[n%3]
                        # overwrite needs s1store of X(n-3) done
                        g.wait_ge(prep, pq[n])
                        g.wait_ge(dve, dcp[n])
                        if n >= 2 and st_cnt[n - 2] > 0:
                            g.wait_ge(sdma, 16 * st_cnt[n - 2])
                        g.trigger_dma(count=1)
                # ---- epilogue input DMAs ----
                g.wait_ge(sdma, 16 * st_cnt[NS - 1])
                for j in range(TSH):
                    g.reg_add(urow, r16, j)
                    if j >= 2:
                        g.wait_ge(pe, PTOT + j - 1)  # es1[j%2] WAR
                    g.dma_start(
                        out=es1[j % 2][:, :],
                        in_=s1store[bass.ds(g.snap(urow), 1), :].rearrange(
                            "a (p c) -> (a p) c", p=128
                        ),
                    ).then_inc(edma, 16)

        # ================= SYNC (HWDGE staging/stores) =================
        @block.sync
        def _(sy):
            sy.wait_ge(init, 1)
            sy.dma_start(
                out=XT[:, 0:256].rearrange("p (a c) -> p a c", a=4),
                in_=xtd.rearrange("(a p) t -> p a t", p=128)[:, :, 0:64],
            ).then_inc(xdma, 16)
            for n in range(NS):
                if n + 1 <= t_steps - 1:
                    if n >= 1:
                        sy.wait_ge(pe, pe_g0[n - 1])
                    sy.dma_start(
                        out=XT[:, ((n + 1) % 2) * 256:((n + 1) % 2 + 1) * 256]
                        .rearrange("p (a c) -> p a c", a=4),
                        in_=xtd.rearrange("(a p) t -> p a t", p=128)[
                            :, :, (n + 1) * 64:(n + 2) * 64
                        ],
                    ).then_inc(xdma, 16)
                if 3 <= n <= t_steps + 2:
                    m = n - 1
                    sy.wait_ge(rs[m % 3], rth(m))
                    sy.dma_start(
                        out=s1store[n - 3, :].rearrange(
                            "(p k c) -> p k c", p=128, k=8
                        ),
                        in_=Gb[m % 3].rearrange("p (k c) -> p k c", k=8)[
                            :, :, 64:128
                        ],
                    ).then_inc(sdma, 16)
            for j in range(TSH):
                sy.wait_ge(dve, DTOT + j * 4 + 4)
                sy.dma_start(out=yd[:, j, :], in_=ebuf[:, :]).then_inc(ydma, 16)

        # ================= TENSOR (PE) =================
        @block.tensor
        def _(te):
            te.wait_ge(init, 1)
            for n in range(NS):
                f = flags(n)
                p2 = n % 2
                # --- A+C interleaved pairs (data >= 2 supersteps old) ---
                if f["L1"]:
                    te.wait_ge(rs[(n - 2) % 3], rth(n - 2))
                    if n >= 4 and flags(n - 2)["L1"]:
                        te.wait_ge(acts, ag1[n - 2])  # ps_g1[p2] WAR
                if f["L0"]:
                    te.wait_ge(xdma, 16 * (xd_cnt[n - 1] if n >= 1 else 1))
                    if n >= 2 and flags(n - 2)["L0"]:
                        te.wait_ge(acts, ag0[n - 2])  # ps_g0[p2] WAR
                mm_g0 = None
                for k in range(8):
                    if f["L1"]:
                        te.matmul(
                            ps_g1[p2][:, :],
                            Gb[(n - 2) % 3][:, k * 128:k * 128 + 64],
                            W1[:, k * 512:(k + 1) * 512],
                            start=(k == 0), stop=False,
                        )
                    if f["L0"] and k < 4:
                        te.matmul(
                            ps_g0[p2][:, :],
                            XT[:, p2 * 256 + k * 64:p2 * 256 + (k + 1) * 64],
                            W0[:, k * 512:(k + 1) * 512],
                            start=(k == 0), stop=False,
                        )
                    if f["L0"] and k == 4:
                        mm_g0 = te.matmul(
                            ps_g0[p2][:, :], ONES[:, :], W0[:, 12 * 512:13 * 512],
                            start=False, stop=(not f["L0dep"]),
                        )
                # keep-warm: the HAM clock gate only promotes to 2.4GHz
                # after ~3.4us of SUSTAINED PE busy, so fill the exchange
                # flight window with back-to-back dummy matmuls (unused
                # partition half of the gates1 PSUM bank).  A/C (~3us) plus
                # these (~2us) end just before the typical arrival, so the
                # D/E burst starts on a warm clock.
                if f["L0dep"] or f["Edep"]:
                    for w in range(5):
                        te.matmul(
                            ps_g1b[p2][0:64, :], ONES[:, :], W0[:, 0:512],
                            start=True, stop=True, skip_group_check=True,
                        )
                    for w in range(4):
                        te.wait_ge(
                            rs[(n - 1) % 3], max(1, rth(n - 1) - 14 + 4 * w)
                        )
                        te.matmul(
                            ps_g1b[p2][0:64, :], ONES[:, :], W0[:, 0:512],
                            start=True, stop=True, skip_group_check=True,
                        )
                # --- D+E pairs: both consume exchange n-1 ---
                if f["L0dep"] or f["Edep"]:
                    te.wait_ge(rs[(n - 1) % 3], rth(n - 1))
                for k in range(8):
                    if f["L0dep"]:
                        mm_g0 = te.matmul(
                            ps_g0[p2][:, :],
                            Gb[(n - 1) % 3][:, k * 128:k * 128 + 64],
                            W0[:, (4 + k) * 512:(5 + k) * 512],
                            start=False, stop=(k == 7),
                        )
                    if f["Edep"]:
                        te.matmul(
                            ps_g1[p2][:, :],
                            Gb[(n - 1) % 3][:, k * 128 + 64:(k + 1) * 128],
                            W1[:, (8 + k) * 512:(9 + k) * 512],
                            start=False, stop=False,
                        )
                if f["L0"]:
                    mm_g0.then_inc(pe, 1)
                if f["L1"]:
                    te.matmul(
                        ps_g1[p2][:, :], ONES[:, :], W1[:, 16 * 512:17 * 512],
                        start=False, stop=True,
                    ).then_inc(pe, 1)
                # --- T: one full transpose of this superstep's s pair ---
                if f["X"]:
                    if f["L1"]:
                        te.wait_ge(dve, ds1[n])
                    elif f["L0"]:
                        te.wait_ge(dve, ds0[n])
                    if n >= 2:
                        te.wait_ge(dve, dcp[n - 2])  # ps_t[p2] WAR
                    te.transpose(
                        ps_t[p2][:, :], sS[p2][:, :], IDN[:, :],
                    ).then_inc(pe, 1)
            # ---- epilogue GEMMs ----
            for j in range(TSH):
                # keep the PE clock warm across the per-step DMA waits
                for w in range(3):
                    te.matmul(
                        ps_g1b[j % 2][0:64, :], ONES[:, :], W0[:, 0:512],
                        start=True, stop=True, skip_group_check=True,
                    )
                te.wait_ge(edma, 16 * (j + 1))
                if j >= 2:
                    te.wait_ge(acts, ATOT + j * 2 - 2)  # ps_e WAR
                mm_e = None
                for k in range(8):
                    mm_e = te.matmul(
                        ps_e[j % 2][:, :],
                        es1[j % 2][:, k * 64:(k + 1) * 64],
                        P1S[:, k * 512:(k + 1) * 512],
                        start=(k == 0), stop=(k == 7),
                    )
                mm_e.then_inc(pe, 1)

        # ================= SCALAR (ACT) =================
        @block.scalar
        def _(sc):
            for n in range(NS):
                f = flags(n)
                p2 = n % 2
                # gate order [i, f, o, g]: one sigmoid + one tanh per layer
                if f["L0"]:
                    sc.wait_ge(pe, pe_g0[n])
                    sc.activation(actb[0:64, 0:384], ps_g0[p2][:, 0:384], AF.Sigmoid)
                    sc.activation(
                        actb[0:64, 384:512], ps_g0[p2][:, 384:512], AF.Tanh
                    ).then_inc(acts, 1)
                if f["L1"]:
                    sc.wait_ge(pe, pe_g1[n])
                    sc.activation(actb[64:128, 0:384], ps_g1[p2][:, 0:384], AF.Sigmoid)
                    sc.activation(
                        actb[64:128, 384:512], ps_g1[p2][:, 384:512], AF.Tanh
                    ).then_inc(acts, 1)
                if f["L0"]:
                    sc.wait_ge(dve, dc0[n])
                    sc.activation(
                        thc[0:64, 0:128], cbuf[0:64, 0:128], AF.Tanh
                    ).then_inc(acts, 1)
                if f["L1"]:
                    sc.wait_ge(dve, dc1[n])
                    sc.activation(
                        thc[64:128, 0:128], cbuf[64:128, 0:128], AF.Tanh
                    ).then_inc(acts, 1)
            for j in range(TSH):
                sc.wait_ge(dve, DTOT + j * 4 + 1)
                if j >= 1:
                    sc.wait_ge(ydma, 16 * j)  # ebuf WAR vs output DMA
                sc.activation(
                    emx[:, 1:2], emx[:, 0:1], AF.Copy, scale=-1.0
                ).then_inc(acts, 1)
                sc.wait_ge(acts, ATOT + j * 2 + 1)
                sc.activation(
                    ebuf[:, :], ps_e[j % 2][:, :], AF.Exp, bias=emx[:, 1:2]
                ).then_inc(acts, 1)

        # ================= VECTOR (DVE) =================
        @block.vector
        def _(ve):
            for n in range(NS):
                f = flags(n)
                p2 = n % 2
                # layout [i, f, o, g]: c = f*c + i*tanh(g); s = o*tanh(c)
                if f["L0"]:
                    ve.wait_ge(acts, ag0[n])
                    ve.tensor_mul(cbuf[0:64, :], actb[0:64, 128:256], cbuf[0:64, :])
                    ve.tensor_mul(actb[0:64, 0:128], actb[0:64, 0:128], actb[0:64, 384:512])
                    ve.tensor_add(
                        cbuf[0:64, :], cbuf[0:64, :], actb[0:64, 0:128]
                    ).then_inc(dve, 1)
                if f["L1"]:
                    ve.wait_ge(acts, ag1[n])
                    ve.tensor_mul(cbuf[64:128, :], actb[64:128, 128:256], cbuf[64:128, :])
                    ve.tensor_mul(actb[64:128, 0:128], actb[64:128, 0:128], actb[64:128, 384:512])
                    ve.tensor_add(
                        cbuf[64:128, :], cbuf[64:128, :], actb[64:128, 0:128]
                    ).then_inc(dve, 1)
                if f["L0"]:
                    ve.wait_ge(acts, at0[n])
                    ve.tensor_mul(
                        sS[p2][0:64, :], actb[0:64, 256:384], thc[0:64, :]
                    ).then_inc(dve, 1)
                if f["L1"]:
                    ve.wait_ge(acts, at1[n])
                    ve.tensor_mul(
                        sS[p2][64:128, :], actb[64:128, 256:384], thc[64:128, :]
                    ).then_inc(dve, 1)
                if f["X"]:
                    ve.wait_ge(pe, pe_t[n])
                    if n >= 2:
                        ve.wait_ge(ls, 16 * cnt_x(n - 2))  # SS[p2] WAR
                    ve.tensor_copy(SS[p2][:, :], ps_t[p2][:, :]).then_inc(dve, 1)
            dbase = DTOT
            for j in range(TSH):
                ve.wait_ge(pe, PTOT + j + 1)
                if j >= 1:
                    ve.wait_ge(acts, ATOT + j * 2 - 1)
                ve.tensor_reduce(
                    emx[:, 0:1], ps_e[j % 2][:, :],
                    mybir.AxisListType.X, mybir.AluOpType.max,
                ).then_inc(dve, 1)
                ve.wait_ge(acts, ATOT + j * 2 + 2)
                ve.tensor_reduce(
                    emx[:, 4:5], ebuf[:, :],
                    mybir.AxisListType.X, mybir.AluOpType.add,
                ).then_inc(dve, 1)
                ve.wait_ge(dve, dbase + j * 4 + 2)
                ve.reciprocal(emx[:, 2:3], emx[:, 4:5]).then_inc(dve, 1)
                ve.wait_ge(dve, dbase + j * 4 + 3)
                ve.tensor_scalar_mul(
                    ebuf[:, :], ebuf[:, :], emx[:, 2:3]
                ).then_inc(dve, 1)

    nc.compile()
    return nc


def _prep_inputs(inputs, t_steps=T):
    bf = ml_dtypes.bfloat16
    images = np.asarray(inputs["images"], np.float32)
    captions = np.asarray(inputs["captions"])
    table = np.asarray(inputs["embed_table"], np.float32)
    W_ih = np.asarray(inputs["W_ih"], np.float32)
    W_hh = np.asarray(inputs["W_hh"], np.float32)
    W_hr = np.asarray(inputs["W_hr"], np.float32)
    bsum = (np.asarray(inputs["b_ih"], np.float32)
            + np.asarray(inputs["b_hh"], np.float32))

    P0, P1 = W_hr[0], W_hr[1]
    M00 = W_hh[0] @ P0
    M10 = W_ih[1] @ P0
    M11 = W_hh[1] @ P1

    emb = table[captions[:, :-1]]
    X = np.concatenate([images, emb], axis=1)  # [B, T, E]
    xT = np.ascontiguousarray(
        X.transpose(2, 1, 0)[:, :t_steps, :].reshape(E, t_steps * B)
    ).astype(bf)

    ones = np.zeros((128, 64), bf)
    ones[0, :] = 1
    iden = np.eye(128, dtype=np.float32).astype(bf)
    p1w = np.ascontiguousarray(P1.T.reshape(8, 128, 512)).astype(bf)

    in_maps = []
    for r in range(NCORES):
        # gate row order [i, f, o, g] so one sigmoid covers cols 0:384
        rows = np.concatenate(
            [np.arange(g * 1024 + r * 128, g * 1024 + (r + 1) * 128)
             for g in (0, 1, 3, 2)]
        )
        w0 = np.zeros((13, 128, 512), bf)
        w0[0:4] = W_ih[0][rows].T.reshape(4, 128, 512).astype(bf)
        w0[4:12] = M00[rows].T.reshape(8, 128, 512).astype(bf)
        bt = np.zeros((128, 512), np.float32)
        bt[0, :] = bsum[0][rows]
        w0[12] = bt.astype(bf)
        w1 = np.zeros((17, 128, 512), bf)
        w1[0:8] = M10[rows].T.reshape(8, 128, 512).astype(bf)
        w1[8:16] = M11[rows].T.reshape(8, 128, 512).astype(bf)
        bt1 = np.zeros((128, 512), np.float32)
        bt1[0, :] = bsum[1][rows]
        w1[16] = bt1.astype(bf)
        in_maps.append({
            "w0": w0, "w1": w1, "p1w": p1w, "xT": xT,
            "ones": ones, "iden": iden,
            "rank": np.array([[r]], np.int32),
            "rank16": np.array([[r * (t_steps // NCORES)]], np.int32),
        })
    return in_maps


def kernel(**inputs):
    global LAST_EXEC_NS
    if TRACE:
        _install_trace_hook()
    if "nc" not in _CACHE:
        _CACHE["nc"] = build(T)
    nc = _CACHE["nc"]
    in_maps = _prep_inputs(inputs)
    res = run_bass_kernel_spmd(
        nc, in_maps, core_ids=list(range(8)), trace=TRACE
    )
    LAST_EXEC_NS = res.exec_time_ns
    out = np.concatenate([res.results[r]["y"] for r in range(8)], axis=1)
    return out.astype(np.float32)


if __name__ == "__main__":
    pass


def debug_run(inputs, t_steps=8):
    if TRACE:
        _install_trace_hook()
    nc = build(t_steps, dump=True)
    in_maps = _prep_inputs(inputs, t_steps)
    res = run_bass_kernel_spmd(nc, in_maps, core_ids=list(range(8)), trace=TRACE)
    y = np.concatenate([res.results[r]["y"] for r in range(8)], axis=1)
    s1d = [res.results[r]["s1store"] for r in range(8)]
    _CACHE["dbg"] = [res.results[r].get("dbg") for r in range(8)]
    return y.astype(np.float32), s1d, res.exec_time_ns


# revision 35
# speedup vs baseline: 1.0108x; 1.0108x over previous
"""CNN-LSTM Trainium2 kernel: 8-way tensor-parallel over the 4H gate dim.

Strategy (v2):
- Host folds the hidden projection into the gate weights (M00 = W_hh0 @ W_hr0,
  M10 = W_ih1 @ W_hr0, M11 = W_hh1 @ W_hr1) so the recurrence runs entirely on
  the sharded s = sigmoid(o)*tanh(c) vectors (H=1024, 128 per core).
- The two layers' gate GEMMs run CONCURRENTLY in the two halves of the PE
  array via column tiling: gates0 PSUM sits at partitions 0:64 (col-group
  0-1), gates1 at 64:128 (col-group 2-3); interleaved matmul pairs execute
  with ~4ns stagger.  All L1 elementwise state lives on partitions 64:128.
- Gate row order is [i, f, o, g] so each layer needs one sigmoid op over
  [*, 0:384] and one tanh over [*, 384:512].
- The s-exchange is SPLIT: X0(n) broadcasts s0T[n] (16KB) as soon as the L0
  chain produces it mid-superstep; X1(n) broadcasts s1T[n-2] later.  Each has
  its own rsem set (rs0/rs1 x3 buffers) and fires via its own trigger, giving
  every consumer nearly a full superstep of flight slack and taking the L1
  chain off the critical cycle.
- te stream order [.. D/E(n) pairs, T0(n), T1(n), A/C(n+1) pairs ..] keeps PE
  idle gaps under the ~3.4us HAM re-throttle window.
- Epilogue: h1 = P1 @ s1 + softmax, sharded over T (16 steps/core).
"""
import sys
import os
import numpy as np

sys.path.insert(0, "/opt/trn_rl_repo")

import concourse.bass as bass  # noqa: E402
import concourse.bacc as bacc  # noqa: E402
import concourse.mybir as mybir  # noqa: E402
from concourse.bass_utils import run_bass_kernel_spmd  # noqa: E402
import ml_dtypes  # noqa: E402

BF = mybir.dt.bfloat16
F32 = mybir.dt.float32
AF = mybir.ActivationFunctionType

B, T, E, H, V = 64, 128, 512, 1024, 10000
NCORES = 8
TRACE = False
LAST_EXEC_NS = None
_CACHE = {}


def _install_trace_hook():
    import types, contextlib, ctypes

    if "antenv.axon_hooks" in sys.modules:
        return
    mod = types.ModuleType("antenv.axon_hooks")
    mod._hook = None
    mod.set_axon_ntff_profile_hook = lambda h: setattr(mod, "_hook", h)
    mod.get_axon_ntff_profile_hook = lambda: mod._hook
    sys.modules["antenv.axon_hooks"] = mod
    import antenv

    antenv.axon_hooks = mod
    so_path = "/opt/axon/libaxon_pjrt.so"
    try:
        lib = ctypes.CDLL(so_path)
    except OSError:
        return
    if not hasattr(lib, "axon_start_nrt_profile"):
        return
    lib.axon_start_nrt_profile.argtypes = [ctypes.POINTER(ctypes.c_int64), ctypes.c_size_t]
    lib.axon_start_nrt_profile.restype = ctypes.c_int64
    lib.axon_stop_nrt_profile.argtypes = [ctypes.c_char_p]
    lib.axon_stop_nrt_profile.restype = ctypes.c_int64

    @contextlib.contextmanager
    def _hook(output_dir, device_ids):
        import jax

        jax.devices()
        if device_ids:
            ids = (ctypes.c_int64 * len(device_ids))(*device_ids)
            rc = lib.axon_start_nrt_profile(ids, len(device_ids))
        else:
            rc = lib.axon_start_nrt_profile(None, 0)
        if rc != 0:
            raise RuntimeError(f"axon_start_nrt_profile rc={rc}")
        try:
            yield
        finally:
            n = lib.axon_stop_nrt_profile(str(output_dir).encode())
            print(f"profile: {n} file(s) -> {output_dir}", file=sys.stderr)

    mod.set_axon_ntff_profile_hook(_hook)


def build(t_steps=T, dump=False):
    NS = t_steps + 3  # supersteps 0 .. t_steps+2
    TSH = t_steps // NCORES  # epilogue steps per core

    nc = bacc.Bacc("TRN2", target_bir_lowering=False, debug=False, num_devices=8,
                   num_swdge_queues=2)

    # ---- I/O ----
    w0d = nc.dram_tensor("w0", [13, 128, 512], BF, kind="ExternalInput")
    w1d = nc.dram_tensor("w1", [17, 128, 512], BF, kind="ExternalInput")
    p1d = nc.dram_tensor("p1w", [8, 128, 512], BF, kind="ExternalInput")
    xtd = nc.dram_tensor("xT", [512, t_steps * 64], BF, kind="ExternalInput")
    onesd = nc.dram_tensor("ones", [128, 64], BF, kind="ExternalInput")
    idend = nc.dram_tensor("iden", [128, 128], BF, kind="ExternalInput")
    rankd = nc.dram_tensor("rank", [1, 1], mybir.dt.int32, kind="ExternalInput")
    rank16d = nc.dram_tensor("rank16", [1, 1], mybir.dt.int32, kind="ExternalInput")
    yd = nc.dram_tensor("y", [64, TSH, 512], F32, kind="ExternalOutput")
    s1store = nc.dram_tensor(
        "s1store", [t_steps, 128 * 512], BF,
        kind="ExternalOutput" if dump else "Internal",
    )

    # ---- SBUF ----
    # L0 state lives on partitions 0:64, L1 state on 64:128 so the two
    # layers' GEMMs run concurrently in the two PE column halves.
    W0 = nc.alloc_sbuf_tensor("W0", [128, 13 * 512], BF)
    W1 = nc.alloc_sbuf_tensor("W1", [128, 17 * 512], BF)
    P1S = nc.alloc_sbuf_tensor("P1S", [128, 8 * 512], BF)
    Gb = [nc.alloc_sbuf_tensor(f"G{q}", [128, 1024], BF) for q in range(3)]
    SS = [nc.alloc_sbuf_tensor(f"SS{p}", [128, 128], BF) for p in range(2)]
    XT = nc.alloc_sbuf_tensor("XT", [128, 2 * 256], BF)
    ONES = nc.alloc_sbuf_tensor("ONES", [128, 64], BF)
    IDN = nc.alloc_sbuf_tensor("IDN", [128, 128], BF)
    actb = nc.alloc_sbuf_tensor("actb", [128, 512], F32)  # i,f,o,g; L0|L1 halves
    cbuf = nc.alloc_sbuf_tensor("cbuf", [128, 128], F32)  # c0 | c1 halves
    thc = nc.alloc_sbuf_tensor("thc", [128, 128], F32)
    sS = [nc.alloc_sbuf_tensor(f"sS{p}", [128, 128], BF) for p in range(2)]
    es1 = [nc.alloc_sbuf_tensor(f"es1_{p}", [128, 512], BF) for p in range(2)]
    emx = nc.alloc_sbuf_tensor("emx", [64, 8], F32)
    ebuf = nc.alloc_sbuf_tensor("ebuf", [64, 512], F32)

    # ---- PSUM (7 banks) ----
    ps_g0 = [nc.alloc_psum_tensor(f"psg0_{p}", [64, 512], F32) for p in range(2)]
    ps_g1b = [nc.alloc_psum_tensor(f"psg1_{p}", [128, 512], F32) for p in range(2)]
    ps_g1 = [t[64:128, :] for t in ps_g1b]  # gates1 at PE col-group 2-3
    ps_t = [nc.alloc_psum_tensor(f"pst_{p}", [128, 128], BF) for p in range(2)]
    ps_e = [nc.alloc_psum_tensor(f"pse_{p}", [64, 512], F32) for p in range(2)]

    # ---- semaphores ----
    rs = [nc.alloc_semaphore(f"rs_{q}") for q in range(3)]
    ls = nc.alloc_semaphore("ls")
    prep = nc.alloc_semaphore("prep")
    pe = nc.alloc_semaphore("pe")
    acts = nc.alloc_semaphore("acts")
    dve = nc.alloc_semaphore("dve")
    xdma = nc.alloc_semaphore("xdma")
    sdma = nc.alloc_semaphore("sdma")
    edma = nc.alloc_semaphore("edma")
    idma = nc.alloc_semaphore("idma")
    init = nc.alloc_semaphore("init")
    ydma = nc.alloc_semaphore("ydma")

    # ---- schedule booleans ----
    def flags(n):
        return dict(
            L0=(n <= t_steps - 1),          # gates0 / s0[n]
            L0dep=(1 <= n <= t_steps - 1),  # D-block
            L1=(2 <= n <= t_steps + 1),     # gates1 / s1[n-2]
            Edep=(3 <= n <= t_steps + 1),   # E-block
            X=(n <= t_steps + 1),           # pair exchange X(n)
        )

    # flat all-to-all pair broadcast: 8 senders x 2 rsem incs per
    # exchange.  X(n) carries [s0T[n] | s1T[n-2]].
    def rth(m):
        return 16 * (m // 3 + 1)

    def cnt_x(m):  # exchanges with index <= m
        return min(m, t_steps + 1) + 1 if m >= 0 else 0

    # ---- analytic milestone tables (cumulative then_inc counts) ----
    pe_g0, pe_g1, pe_t = {}, {}, {}
    ag0, ag1, at0, at1 = {}, {}, {}, {}
    dc0, dc1, ds0, ds1, dcp = {}, {}, {}, {}, {}
    pq, pf = {}, {}
    xd_cnt, st_cnt = {}, {}
    c_pe = c_a = c_d = c_pr = 0
    c_x = 1
    c_st = 0
    for n in range(NS):
        f = flags(n)
        if f["X"]:
            c_pr += 1
        pq[n] = c_pr
        if f["L0"]:
            c_pe += 1
        pe_g0[n] = c_pe
        if f["L1"]:
            c_pe += 1
        pe_g1[n] = c_pe
        if f["X"]:
            c_pe += 1
        pe_t[n] = c_pe
        if f["L0"]:
            c_a += 1
        ag0[n] = c_a
        if f["L1"]:
            c_a += 1
        ag1[n] = c_a
        if f["L0"]:
            c_a += 1
        at0[n] = c_a
        if f["L1"]:
            c_a += 1
        at1[n] = c_a
        if f["L0"]:
            c_d += 1
        dc0[n] = c_d
        if f["L1"]:
            c_d += 1
        dc1[n] = c_d
        if f["L0"]:
            c_d += 1
        ds0[n] = c_d
        if f["L1"]:
            c_d += 1
        ds1[n] = c_d
        if f["X"]:
            c_d += 1
        dcp[n] = c_d
        if n + 1 <= t_steps - 1:
            c_x += 1
        xd_cnt[n] = c_x
        if 3 <= n <= t_steps + 2:
            c_st += 1
        st_cnt[n] = c_st
    PTOT, ATOT, DTOT = c_pe, c_a, c_d

    with nc.Block() as block:

        # ================= GPSIMD =================
        @block.gpsimd
        def _(g):
            with g.register("rank") as rank, g.register("urow") as urow, \
                    g.register("r16") as r16:
                g.load(rank, rankd.ap())
                g.load(r16, rank16d.ap())
                g.dma_start(
                    out=W0.rearrange("p (k c) -> p k c", k=13),
                    in_=w0d.rearrange("k p c -> p k c"),
                ).then_inc(idma, 16)
                g.dma_start(
                    out=W1.rearrange("p (k c) -> p k c", k=17),
                    in_=w1d.rearrange("k p c -> p k c"),
                ).then_inc(idma, 16)
                g.dma_start(
                    out=P1S.rearrange("p (k c) -> p k c", k=8),
                    in_=p1d.rearrange("k p c -> p k c"),
                ).then_inc(idma, 16)
                g.dma_start(out=ONES[:, :], in_=onesd[:, :]).then_inc(idma, 16)
                g.dma_start(out=IDN[:, :], in_=idend[:, :]).then_inc(idma, 16)
                g.wait_ge(idma, 80)
                g.memset(cbuf[:, :], 0.0)
                g.memset(SS[0][:, :], 0.0)
                g.memset(SS[1][:, :], 0.0)
                g.memset(sS[0][:, :], 0.0)
                g.memset(sS[1][:, :], 0.0)
                g.memset(emx[:, :], 0.0).then_inc(init, 1)
                g.bir_kernel_barrier_wait([list(range(8))])
                rdests8 = [(0, k) for k in range(8)]
                for n in range(NS):
                    f = flags(n)
                    if f["X"]:
                        for r in range(8):
                            with g.If_eq(rank, r):
                                g.remote_dma_broadcast(
                                    out_ap=Gb[n % 3][:, r * 128:(r + 1) * 128],
                                    in_ap=SS[n % 2][:, :],
                                    remote_sem=rs[n % 3],
                                    local_sem=ls,
                                    rdests=rdests8,
                                ).then_inc(prep, 1)
                        # fire once the pair copy landed in SS; Gb[n%3]
                        # overwrite needs s1store of X(n-3) done
                        g.wait_ge(prep, pq[n])
                        g.wait_ge(dve, dcp[n])
                        if n >= 2 and st_cnt[n - 2] > 0:
                            g.wait_ge(sdma, 16 * st_cnt[n - 2])
                        g.trigger_dma(count=1)
                # ---- epilogue input DMAs ----
                g.wait_ge(sdma, 16 * st_cnt[NS - 1])
                for j in range(TSH):
                    g.reg_add(urow, r16, j)
                    if j >= 2:
                        g.wait_ge(pe, PTOT + j - 1)  # es1[j%2] WAR
                    g.dma_start(
                        out=es1[j % 2][:, :],
                        in_=s1store[bass.ds(g.snap(urow), 1), :].rearrange(
                            "a (p c) -> (a p) c", p=128
                        ),
                    ).then_inc(edma, 16)

        # ================= SYNC (HWDGE staging/stores) =================
        @block.sync
        def _(sy):
            sy.wait_ge(init, 1)
            sy.dma_start(
                out=XT[:, 0:256].rearrange("p (a c) -> p a c", a=4),
                in_=xtd.rearrange("(a p) t -> p a t", p=128)[:, :, 0:64],
            ).then_inc(xdma, 16)
            for n in range(NS):
                if n + 1 <= t_steps - 1:
                    if n >= 1:
                        sy.wait_ge(pe, pe_g0[n - 1])
                    sy.dma_start(
                        out=XT[:, ((n + 1) % 2) * 256:((n + 1) % 2 + 1) * 256]
                        .rearrange("p (a c) -> p a c", a=4),
                        in_=xtd.rearrange("(a p) t -> p a t", p=128)[
                            :, :, (n + 1) * 64:(n + 2) * 64
                        ],
                    ).then_inc(xdma, 16)
                if 3 <= n <= t_steps + 2:
                    m = n - 1
                    sy.wait_ge(rs[m % 3], rth(m))
                    sy.dma_start(
                        out=s1store[n - 3, :].rearrange(
                            "(p k c) -> p k c", p=128, k=8
                        ),
                        in_=Gb[m % 3].rearrange("p (k c) -> p k c", k=8)[
                            :, :, 64:128
                        ],
                    ).then_inc(sdma, 16)
            for j in range(TSH):
                sy.wait_ge(dve, DTOT + j * 4 + 4)
                sy.dma_start(out=yd[:, j, :], in_=ebuf[:, :]).then_inc(ydma, 16)

        # ================= TENSOR (PE) =================
        @block.tensor
        def _(te):
            te.wait_ge(init, 1)
            for n in range(NS):
                f = flags(n)
                p2 = n % 2
                # --- A+C interleaved pairs (data >= 2 supersteps old) ---
                if f["L1"]:
                    te.wait_ge(rs[(n - 2) % 3], rth(n - 2))
                    if n >= 4 and flags(n - 2)["L1"]:
                        te.wait_ge(acts, ag1[n - 2])  # ps_g1[p2] WAR
                if f["L0"]:
                    te.wait_ge(xdma, 16 * (xd_cnt[n - 1] if n >= 1 else 1))
                    if n >= 2 and flags(n - 2)["L0"]:
                        te.wait_ge(acts, ag0[n - 2])  # ps_g0[p2] WAR
                mm_g0 = None
                for k in range(8):
                    if f["L1"]:
                        te.matmul(
                            ps_g1[p2][:, :],
                            Gb[(n - 2) % 3][:, k * 128:k * 128 + 64],
                            W1[:, k * 512:(k + 1) * 512],
                            start=(k == 0), stop=False,
                        )
                    if f["L0"] and k < 4:
                        te.matmul(
                            ps_g0[p2][:, :],
                            XT[:, p2 * 256 + k * 64:p2 * 256 + (k + 1) * 64],
                            W0[:, k * 512:(k + 1) * 512],
                            start=(k == 0), stop=False,
                        )
                    if f["L0"] and k == 4:
                        mm_g0 = te.matmul(
                            ps_g0[p2][:, :], ONES[:, :], W0[:, 12 * 512:13 * 512],
                            start=False, stop=(not f["L0dep"]),
                        )
                # keep-warm: the HAM clock gate only promotes to 2.4GHz
                # after ~3.4us of SUSTAINED PE busy, so fill the exchange
                # flight window with back-to-back dummy matmuls (unused
                # partition half of the gates1 PSUM bank).  A/C (~3us) plus
                # these (~2us) end just before the typical arrival, so the
                # D/E burst starts on a warm clock.
                if f["L0dep"] or f["Edep"]:
                    for w in range(4):
                        te.matmul(
                            ps_g1b[p2][0:64, :], ONES[:, :], W0[:, 0:512],
                            start=True, stop=True, skip_group_check=True,
                        )
                    for w in range(4):
                        te.wait_ge(
                            rs[(n - 1) % 3], max(1, rth(n - 1) - 14 + 4 * w)
                        )
                        te.matmul(
                            ps_g1b[p2][0:64, :], ONES[:, :], W0[:, 0:512],
                            start=True, stop=True, skip_group_check=True,
                        )
                # --- D+E pairs: both consume exchange n-1 ---
                if f["L0dep"] or f["Edep"]:
                    te.wait_ge(rs[(n - 1) % 3], rth(n - 1))
                for k in range(8):
                    if f["L0dep"]:
                        mm_g0 = te.matmul(
                            ps_g0[p2][:, :],
                            Gb[(n - 1) % 3][:, k * 128:k * 128 + 64],
                            W0[:, (4 + k) * 512:(5 + k) * 512],
                            start=False, stop=(k == 7),
                        )
                    if f["Edep"]:
                        te.matmul(
                            ps_g1[p2][:, :],
                            Gb[(n - 1) % 3][:, k * 128 + 64:(k + 1) * 128],
                            W1[:, (8 + k) * 512:(9 + k) * 512],
                            start=False, stop=False,
                        )
                if f["L0"]:
                    mm_g0.then_inc(pe, 1)
                if f["L1"]:
                    te.matmul(
                        ps_g1[p2][:, :], ONES[:, :], W1[:, 16 * 512:17 * 512],
                        start=False, stop=True,
                    ).then_inc(pe, 1)
                # --- T: one full transpose of this superstep's s pair ---
                if f["X"]:
                    if f["L1"]:
                        te.wait_ge(dve, ds1[n])
                    elif f["L0"]:
                        te.wait_ge(dve, ds0[n])
                    if n >= 2:
                        te.wait_ge(dve, dcp[n - 2])  # ps_t[p2] WAR
                    te.transpose(
                        ps_t[p2][:, :], sS[p2][:, :], IDN[:, :],
                    ).then_inc(pe, 1)
            # ---- epilogue GEMMs ----
            for j in range(TSH):
                te.wait_ge(edma, 16 * (j + 1))
                if j >= 2:
                    te.wait_ge(acts, ATOT + j * 2 - 2)  # ps_e WAR
                mm_e = None
                for k in range(8):
                    mm_e = te.matmul(
                        ps_e[j % 2][:, :],
                        es1[j % 2][:, k * 64:(k + 1) * 64],
                        P1S[:, k * 512:(k + 1) * 512],
                        start=(k == 0), stop=(k == 7),
                    )
                mm_e.then_inc(pe, 1)

        # ================= SCALAR (ACT) =================
        @block.scalar
        def _(sc):
            for n in range(NS):
                f = flags(n)
                p2 = n % 2
                # gate order [i, f, o, g]: one sigmoid + one tanh per layer
                if f["L0"]:
                    sc.wait_ge(pe, pe_g0[n])
                    sc.activation(actb[0:64, 0:384], ps_g0[p2][:, 0:384], AF.Sigmoid)
                    sc.activation(
                        actb[0:64, 384:512], ps_g0[p2][:, 384:512], AF.Tanh
                    ).then_inc(acts, 1)
                if f["L1"]:
                    sc.wait_ge(pe, pe_g1[n])
                    sc.activation(actb[64:128, 0:384], ps_g1[p2][:, 0:384], AF.Sigmoid)
                    sc.activation(
                        actb[64:128, 384:512], ps_g1[p2][:, 384:512], AF.Tanh
                    ).then_inc(acts, 1)
                if f["L0"]:
                    sc.wait_ge(dve, dc0[n])
                    sc.activation(
                        thc[0:64, 0:128], cbuf[0:64, 0:128], AF.Tanh
                    ).then_inc(acts, 1)
                if f["L1"]:
                    sc.wait_ge(dve, dc1[n])
                    sc.activation(
                        thc[64:128, 0:128], cbuf[64:128, 0:128], AF.Tanh
                    ).then_inc(acts, 1)
            for j in range(TSH):
                sc.wait_ge(dve, DTOT + j * 4 + 1)
                if j >= 1:
                    sc.wait_ge(ydma, 16 * j)  # ebuf WAR vs output DMA
                sc.activation(
                    emx[:, 1:2], emx[:, 0:1], AF.Copy, scale=-1.0
                ).then_inc(acts, 1)
                sc.wait_ge(acts, ATOT + j * 2 + 1)
                sc.activation(
                    ebuf[:, :], ps_e[j % 2][:, :], AF.Exp, bias=emx[:, 1:2]
                ).then_inc(acts, 1)

        # ================= VECTOR (DVE) =================
        @block.vector
        def _(ve):
            for n in range(NS):
                f = flags(n)
                p2 = n % 2
                # layout [i, f, o, g]: c = f*c + i*tanh(g); s = o*tanh(c)
                if f["L0"]:
                    ve.wait_ge(acts, ag0[n])
                    ve.tensor_mul(cbuf[0:64, :], actb[0:64, 128:256], cbuf[0:64, :])
                    ve.tensor_mul(actb[0:64, 0:128], actb[0:64, 0:128], actb[0:64, 384:512])
                    ve.tensor_add(
                        cbuf[0:64, :], cbuf[0:64, :], actb[0:64, 0:128]
                    ).then_inc(dve, 1)
                if f["L1"]:
                    ve.wait_ge(acts, ag1[n])
                    ve.tensor_mul(cbuf[64:128, :], actb[64:128, 128:256], cbuf[64:128, :])
                    ve.tensor_mul(actb[64:128, 0:128], actb[64:128, 0:128], actb[64:128, 384:512])
                    ve.tensor_add(
                        cbuf[64:128, :], cbuf[64:128, :], actb[64:128, 0:128]
                    ).then_inc(dve, 1)
                if f["L0"]:
                    ve.wait_ge(acts, at0[n])
                    ve.tensor_mul(
                        sS[p2][0:64, :], actb[0:64, 256:384], thc[0:64, :]
                    ).then_inc(dve, 1)
                if f["L1"]:
                    ve.wait_ge(acts, at1[n])
                    ve.tensor_mul(
                        sS[p2][64:128, :], actb[64:128, 256:384], thc[64:128, :]
                    ).then_inc(dve, 1)
                if f["X"]:
                    ve.wait_ge(pe, pe_t[n])
                    if n >= 2:
                        ve.wait_ge(ls, 16 * cnt_x(n - 2))  # SS[p2] WAR
                    ve.tensor_copy(SS[p2][:, :], ps_t[p2][:, :]).then_inc(dve, 1)
            dbase = DTOT
            for j in range(TSH):
                ve.wait_ge(pe, PTOT + j + 1)
                if j >= 1:
                    ve.wait_ge(acts, ATOT + j * 2 - 1)
                ve.tensor_reduce(
                    emx[:, 0:1], ps_e[j % 2][:, :],
                    mybir.AxisListType.X, mybir.AluOpType.max,
                ).then_inc(dve, 1)
                ve.wait_ge(acts, ATOT + j * 2 + 2)
                ve.tensor_reduce(
                    emx[:, 4:5], ebuf[:, :],
                    mybir.AxisListType.X, mybir.AluOpType.add,
                ).then_inc(dve, 1)
                ve.wait_ge(dve, dbase + j * 4 + 2)
                ve.reciprocal(emx[:, 2:3], emx[:, 4:5]).then_inc(dve, 1)
                ve.wait_ge(dve, dbase + j * 4 + 3)
                ve.tensor_scalar_mul(
                    ebuf[:, :], ebuf[:, :], emx[:, 2:3]
                ).then_inc(dve, 1)

    nc.compile()
    return nc


def _prep_inputs(inputs, t_steps=T):
    bf = ml_dtypes.bfloat16
    images = np.asarray(inputs["images"], np.float32)
    captions = np.asarray(inputs["captions"])
    table = np.asarray(inputs["embed_table"], np.float32)
    W_ih = np.asarray(inputs["W_ih"], np.float32)
    W_hh = np.asarray(inputs["W_hh"], np.float32)
    W_hr = np.asarray(inputs["W_hr"], np.float32)
    bsum = (np.asarray(inputs["b_ih"], np.float32)
            + np.asarray(inputs["b_hh"], np.float32))

    P0, P1 = W_hr[0], W_hr[1]
    M00 = W_hh[0] @ P0
    M10 = W_ih[1] @ P0
    M11 = W_hh[1] @ P1

    emb = table[captions[:, :-1]]
    X = np.concatenate([images, emb], axis=1)  # [B, T, E]
    xT = np.ascontiguousarray(
        X.transpose(2, 1, 0)[:, :t_steps, :].reshape(E, t_steps * B)
    ).astype(bf)

    ones = np.zeros((128, 64), bf)
    ones[0, :] = 1
    iden = np.eye(128, dtype=np.float32).astype(bf)
    p1w = np.ascontiguousarray(P1.T.reshape(8, 128, 512)).astype(bf)

    in_maps = []
    for r in range(NCORES):
        # gate row order [i, f, o, g] so one sigmoid covers cols 0:384
        rows = np.concatenate(
            [np.arange(g * 1024 + r * 128, g * 1024 + (r + 1) * 128)
             for g in (0, 1, 3, 2)]
        )
        w0 = np.zeros((13, 128, 512), bf)
        w0[0:4] = W_ih[0][rows].T.reshape(4, 128, 512).astype(bf)
        w0[4:12] = M00[rows].T.reshape(8, 128, 512).astype(bf)
        bt = np.zeros((128, 512), np.float32)
        bt[0, :] = bsum[0][rows]
        w0[12] = bt.astype(bf)
        w1 = np.zeros((17, 128, 512), bf)
        w1[0:8] = M10[rows].T.reshape(8, 128, 512).astype(bf)
        w1[8:16] = M11[rows].T.reshape(8, 128, 512).astype(bf)
        bt1 = np.zeros((128, 512), np.float32)
        bt1[0, :] = bsum[1][rows]
        w1[16] = bt1.astype(bf)
        in_maps.append({
            "w0": w0, "w1": w1, "p1w": p1w, "xT": xT,
            "ones": ones, "iden": iden,
            "rank": np.array([[r]], np.int32),
            "rank16": np.array([[r * (t_steps // NCORES)]], np.int32),
        })
    return in_maps


def kernel(**inputs):
    global LAST_EXEC_NS
    if TRACE:
        _install_trace_hook()
    if "nc" not in _CACHE:
        _CACHE["nc"] = build(T)
    nc = _CACHE["nc"]
    in_maps = _prep_inputs(inputs)
    res = run_bass_kernel_spmd(
        nc, in_maps, core_ids=list(range(8)), trace=TRACE
    )
    LAST_EXEC_NS = res.exec_time_ns
    out = np.concatenate([res.results[r]["y"] for r in range(8)], axis=1)
    return out.astype(np.float32)


if __name__ == "__main__":
    pass


def debug_run(inputs, t_steps=8):
    if TRACE:
        _install_trace_hook()
    nc = build(t_steps, dump=True)
    in_maps = _prep_inputs(inputs, t_steps)
    res = run_bass_kernel_spmd(nc, in_maps, core_ids=list(range(8)), trace=TRACE)
    y = np.concatenate([res.results[r]["y"] for r in range(8)], axis=1)
    s1d = [res.results[r]["s1store"] for r in range(8)]
    _CACHE["dbg"] = [res.results[r].get("dbg") for r in range(8)]
    return y.astype(np.float32), s1d, res.exec_time_ns


# revision 36
# speedup vs baseline: 1.0233x; 1.0123x over previous
"""CNN-LSTM Trainium2 kernel: 8-way tensor-parallel over the 4H gate dim.

Strategy (v2):
- Host folds the hidden projection into the gate weights (M00 = W_hh0 @ W_hr0,
  M10 = W_ih1 @ W_hr0, M11 = W_hh1 @ W_hr1) so the recurrence runs entirely on
  the sharded s = sigmoid(o)*tanh(c) vectors (H=1024, 128 per core).
- The two layers' gate GEMMs run CONCURRENTLY in the two halves of the PE
  array via column tiling: gates0 PSUM sits at partitions 0:64 (col-group
  0-1), gates1 at 64:128 (col-group 2-3); interleaved matmul pairs execute
  with ~4ns stagger.  All L1 elementwise state lives on partitions 64:128.
- Gate row order is [i, f, o, g] so each layer needs one sigmoid op over
  [*, 0:384] and one tanh over [*, 384:512].
- The s-exchange is SPLIT: X0(n) broadcasts s0T[n] (16KB) as soon as the L0
  chain produces it mid-superstep; X1(n) broadcasts s1T[n-2] later.  Each has
  its own rsem set (rs0/rs1 x3 buffers) and fires via its own trigger, giving
  every consumer nearly a full superstep of flight slack and taking the L1
  chain off the critical cycle.
- te stream order [.. D/E(n) pairs, T0(n), T1(n), A/C(n+1) pairs ..] keeps PE
  idle gaps under the ~3.4us HAM re-throttle window.
- Epilogue: h1 = P1 @ s1 + softmax, sharded over T (16 steps/core).
"""
import sys
import os
import numpy as np

sys.path.insert(0, "/opt/trn_rl_repo")

import concourse.bass as bass  # noqa: E402
import concourse.bacc as bacc  # noqa: E402
import concourse.mybir as mybir  # noqa: E402
from concourse.bass_utils import run_bass_kernel_spmd  # noqa: E402
import ml_dtypes  # noqa: E402

BF = mybir.dt.bfloat16
F32 = mybir.dt.float32
AF = mybir.ActivationFunctionType

B, T, E, H, V = 64, 128, 512, 1024, 10000
NCORES = 8
TRACE = False
LAST_EXEC_NS = None
_CACHE = {}


def _install_trace_hook():
    import types, contextlib, ctypes

    if "antenv.axon_hooks" in sys.modules:
        return
    mod = types.ModuleType("antenv.axon_hooks")
    mod._hook = None
    mod.set_axon_ntff_profile_hook = lambda h: setattr(mod, "_hook", h)
    mod.get_axon_ntff_profile_hook = lambda: mod._hook
    sys.modules["antenv.axon_hooks"] = mod
    import antenv

    antenv.axon_hooks = mod
    so_path = "/opt/axon/libaxon_pjrt.so"
    try:
        lib = ctypes.CDLL(so_path)
    except OSError:
        return
    if not hasattr(lib, "axon_start_nrt_profile"):
        return
    lib.axon_start_nrt_profile.argtypes = [ctypes.POINTER(ctypes.c_int64), ctypes.c_size_t]
    lib.axon_start_nrt_profile.restype = ctypes.c_int64
    lib.axon_stop_nrt_profile.argtypes = [ctypes.c_char_p]
    lib.axon_stop_nrt_profile.restype = ctypes.c_int64

    @contextlib.contextmanager
    def _hook(output_dir, device_ids):
        import jax

        jax.devices()
        if device_ids:
            ids = (ctypes.c_int64 * len(device_ids))(*device_ids)
            rc = lib.axon_start_nrt_profile(ids, len(device_ids))
        else:
            rc = lib.axon_start_nrt_profile(None, 0)
        if rc != 0:
            raise RuntimeError(f"axon_start_nrt_profile rc={rc}")
        try:
            yield
        finally:
            n = lib.axon_stop_nrt_profile(str(output_dir).encode())
            print(f"profile: {n} file(s) -> {output_dir}", file=sys.stderr)

    mod.set_axon_ntff_profile_hook(_hook)


def build(t_steps=T, dump=False):
    NS = t_steps + 3  # supersteps 0 .. t_steps+2
    TSH = t_steps // NCORES  # epilogue steps per core

    nc = bacc.Bacc("TRN2", target_bir_lowering=False, debug=False, num_devices=8,
                   num_swdge_queues=2)

    # ---- I/O ----
    w0d = nc.dram_tensor("w0", [13, 128, 512], BF, kind="ExternalInput")
    w1d = nc.dram_tensor("w1", [17, 128, 512], BF, kind="ExternalInput")
    p1d = nc.dram_tensor("p1w", [8, 128, 512], BF, kind="ExternalInput")
    xtd = nc.dram_tensor("xT", [512, t_steps * 64], BF, kind="ExternalInput")
    onesd = nc.dram_tensor("ones", [128, 64], BF, kind="ExternalInput")
    idend = nc.dram_tensor("iden", [128, 128], BF, kind="ExternalInput")
    rankd = nc.dram_tensor("rank", [1, 1], mybir.dt.int32, kind="ExternalInput")
    rank16d = nc.dram_tensor("rank16", [1, 1], mybir.dt.int32, kind="ExternalInput")
    yd = nc.dram_tensor("y", [64, TSH, 512], F32, kind="ExternalOutput")
    s1store = nc.dram_tensor(
        "s1store", [t_steps, 128 * 512], BF,
        kind="ExternalOutput" if dump else "Internal",
    )

    # ---- SBUF ----
    # L0 state lives on partitions 0:64, L1 state on 64:128 so the two
    # layers' GEMMs run concurrently in the two PE column halves.
    W0 = nc.alloc_sbuf_tensor("W0", [128, 13 * 512], BF)
    W1 = nc.alloc_sbuf_tensor("W1", [128, 17 * 512], BF)
    P1S = nc.alloc_sbuf_tensor("P1S", [128, 8 * 512], BF)
    Gb = [nc.alloc_sbuf_tensor(f"G{q}", [128, 1024], BF) for q in range(3)]
    SS = [nc.alloc_sbuf_tensor(f"SS{p}", [128, 128], BF) for p in range(2)]
    XT = nc.alloc_sbuf_tensor("XT", [128, 2 * 256], BF)
    ONES = nc.alloc_sbuf_tensor("ONES", [128, 64], BF)
    IDN = nc.alloc_sbuf_tensor("IDN", [128, 128], BF)
    actb = nc.alloc_sbuf_tensor("actb", [128, 512], F32)  # i,f,o,g; L0|L1 halves
    cbuf = nc.alloc_sbuf_tensor("cbuf", [128, 128], F32)  # c0 | c1 halves
    thc = nc.alloc_sbuf_tensor("thc", [128, 128], F32)
    sS = [nc.alloc_sbuf_tensor(f"sS{p}", [128, 128], BF) for p in range(2)]
    es1 = [nc.alloc_sbuf_tensor(f"es1_{p}", [128, 512], BF) for p in range(2)]
    emx = nc.alloc_sbuf_tensor("emx", [64, 8], F32)
    ebuf = nc.alloc_sbuf_tensor("ebuf", [64, 512], F32)

    # ---- PSUM (7 banks) ----
    ps_g0 = [nc.alloc_psum_tensor(f"psg0_{p}", [64, 512], F32) for p in range(2)]
    ps_g1b = [nc.alloc_psum_tensor(f"psg1_{p}", [128, 512], F32) for p in range(2)]
    ps_g1 = [t[64:128, :] for t in ps_g1b]  # gates1 at PE col-group 2-3
    ps_t = [nc.alloc_psum_tensor(f"pst_{p}", [128, 128], BF) for p in range(2)]
    ps_e = [nc.alloc_psum_tensor(f"pse_{p}", [64, 512], F32) for p in range(2)]

    # ---- semaphores ----
    rs = [nc.alloc_semaphore(f"rs_{q}") for q in range(3)]
    ls = nc.alloc_semaphore("ls")
    prep = nc.alloc_semaphore("prep")
    pe = nc.alloc_semaphore("pe")
    acts = nc.alloc_semaphore("acts")
    dve = nc.alloc_semaphore("dve")
    xdma = nc.alloc_semaphore("xdma")
    sdma = nc.alloc_semaphore("sdma")
    edma = nc.alloc_semaphore("edma")
    idma = nc.alloc_semaphore("idma")
    init = nc.alloc_semaphore("init")
    ydma = nc.alloc_semaphore("ydma")

    # ---- schedule booleans ----
    def flags(n):
        return dict(
            L0=(n <= t_steps - 1),          # gates0 / s0[n]
            L0dep=(1 <= n <= t_steps - 1),  # D-block
            L1=(2 <= n <= t_steps + 1),     # gates1 / s1[n-2]
            Edep=(3 <= n <= t_steps + 1),   # E-block
            X=(n <= t_steps + 1),           # pair exchange X(n)
        )

    # flat all-to-all pair broadcast: 8 senders x 2 rsem incs per
    # exchange.  X(n) carries [s0T[n] | s1T[n-2]].
    def rth(m):
        return 16 * (m // 3 + 1)

    def cnt_x(m):  # exchanges with index <= m
        return min(m, t_steps + 1) + 1 if m >= 0 else 0

    # ---- analytic milestone tables (cumulative then_inc counts) ----
    pe_g0, pe_g1, pe_t = {}, {}, {}
    ag0, ag1, at0, at1 = {}, {}, {}, {}
    dc0, dc1, ds0, ds1, dcp = {}, {}, {}, {}, {}
    pq, pf = {}, {}
    xd_cnt, st_cnt = {}, {}
    c_pe = c_a = c_d = c_pr = 0
    c_x = 1
    c_st = 0
    for n in range(NS):
        f = flags(n)
        if f["X"]:
            c_pr += 1
        pq[n] = c_pr
        if f["L0"]:
            c_pe += 1
        pe_g0[n] = c_pe
        if f["L1"]:
            c_pe += 1
        pe_g1[n] = c_pe
        if f["X"]:
            c_pe += 1
        pe_t[n] = c_pe
        if f["L0"]:
            c_a += 1
        ag0[n] = c_a
        if f["L1"]:
            c_a += 1
        ag1[n] = c_a
        if f["L0"]:
            c_a += 1
        at0[n] = c_a
        if f["L1"]:
            c_a += 1
        at1[n] = c_a
        if f["L0"]:
            c_d += 1
        dc0[n] = c_d
        if f["L1"]:
            c_d += 1
        dc1[n] = c_d
        if f["L0"]:
            c_d += 1
        ds0[n] = c_d
        if f["L1"]:
            c_d += 1
        ds1[n] = c_d
        if f["X"]:
            c_d += 1
        dcp[n] = c_d
        if n + 1 <= t_steps - 1:
            c_x += 1
        xd_cnt[n] = c_x
        if 3 <= n <= t_steps + 2:
            c_st += 1
        st_cnt[n] = c_st
    PTOT, ATOT, DTOT = c_pe, c_a, c_d

    with nc.Block() as block:

        # ================= GPSIMD =================
        @block.gpsimd
        def _(g):
            with g.register("rank") as rank, g.register("urow") as urow, \
                    g.register("r16") as r16:
                g.load(rank, rankd.ap())
                g.load(r16, rank16d.ap())
                g.dma_start(
                    out=W0.rearrange("p (k c) -> p k c", k=13),
                    in_=w0d.rearrange("k p c -> p k c"),
                ).then_inc(idma, 16)
                g.dma_start(
                    out=W1.rearrange("p (k c) -> p k c", k=17),
                    in_=w1d.rearrange("k p c -> p k c"),
                ).then_inc(idma, 16)
                g.dma_start(
                    out=P1S.rearrange("p (k c) -> p k c", k=8),
                    in_=p1d.rearrange("k p c -> p k c"),
                ).then_inc(idma, 16)
                g.dma_start(out=ONES[:, :], in_=onesd[:, :]).then_inc(idma, 16)
                g.dma_start(out=IDN[:, :], in_=idend[:, :]).then_inc(idma, 16)
                g.wait_ge(idma, 80)
                g.memset(cbuf[:, :], 0.0)
                g.memset(SS[0][:, :], 0.0)
                g.memset(SS[1][:, :], 0.0)
                g.memset(sS[0][:, :], 0.0)
                g.memset(sS[1][:, :], 0.0)
                g.memset(emx[:, :], 0.0).then_inc(init, 1)
                g.bir_kernel_barrier_wait([list(range(8))])
                rdests8 = [(0, k) for k in range(8)]
                for n in range(NS):
                    f = flags(n)
                    if f["X"]:
                        for r in range(8):
                            with g.If_eq(rank, r):
                                g.remote_dma_broadcast(
                                    out_ap=Gb[n % 3][:, r * 128:(r + 1) * 128],
                                    in_ap=SS[n % 2][:, :],
                                    remote_sem=rs[n % 3],
                                    local_sem=ls,
                                    rdests=rdests8,
                                ).then_inc(prep, 1)
                        # fire once the pair copy landed in SS; Gb[n%3]
                        # overwrite needs s1store of X(n-3) done
                        g.wait_ge(prep, pq[n])
                        # fire off the TRANSPOSE milestone: the DMA engines
                        # only read SS ~0.8-1.0us after the trigger executes,
                        # while the DVE copy lands ~0.4us after the transpose
                        # -- the copy hides inside the DMA startup latency.
                        g.wait_ge(pe, pe_t[n])
                        if n >= 2 and st_cnt[n - 2] > 0:
                            g.wait_ge(sdma, 16 * st_cnt[n - 2])
                        g.trigger_dma(count=1)
                # ---- epilogue input DMAs ----
                g.wait_ge(sdma, 16 * st_cnt[NS - 1])
                for j in range(TSH):
                    g.reg_add(urow, r16, j)
                    if j >= 2:
                        g.wait_ge(pe, PTOT + j - 1)  # es1[j%2] WAR
                    g.dma_start(
                        out=es1[j % 2][:, :],
                        in_=s1store[bass.ds(g.snap(urow), 1), :].rearrange(
                            "a (p c) -> (a p) c", p=128
                        ),
                    ).then_inc(edma, 16)

        # ================= SYNC (HWDGE staging/stores) =================
        @block.sync
        def _(sy):
            sy.wait_ge(init, 1)
            sy.dma_start(
                out=XT[:, 0:256].rearrange("p (a c) -> p a c", a=4),
                in_=xtd.rearrange("(a p) t -> p a t", p=128)[:, :, 0:64],
            ).then_inc(xdma, 16)
            for n in range(NS):
                if n + 1 <= t_steps - 1:
                    if n >= 1:
                        sy.wait_ge(pe, pe_g0[n - 1])
                    sy.dma_start(
                        out=XT[:, ((n + 1) % 2) * 256:((n + 1) % 2 + 1) * 256]
                        .rearrange("p (a c) -> p a c", a=4),
                        in_=xtd.rearrange("(a p) t -> p a t", p=128)[
                            :, :, (n + 1) * 64:(n + 2) * 64
                        ],
                    ).then_inc(xdma, 16)
                if 3 <= n <= t_steps + 2:
                    m = n - 1
                    sy.wait_ge(rs[m % 3], rth(m))
                    sy.dma_start(
                        out=s1store[n - 3, :].rearrange(
                            "(p k c) -> p k c", p=128, k=8
                        ),
                        in_=Gb[m % 3].rearrange("p (k c) -> p k c", k=8)[
                            :, :, 64:128
                        ],
                    ).then_inc(sdma, 16)
            for j in range(TSH):
                sy.wait_ge(dve, DTOT + j * 4 + 4)
                sy.dma_start(out=yd[:, j, :], in_=ebuf[:, :]).then_inc(ydma, 16)

        # ================= TENSOR (PE) =================
        @block.tensor
        def _(te):
            te.wait_ge(init, 1)
            for n in range(NS):
                f = flags(n)
                p2 = n % 2
                # --- A+C interleaved pairs (data >= 2 supersteps old) ---
                if f["L1"]:
                    te.wait_ge(rs[(n - 2) % 3], rth(n - 2))
                    if n >= 4 and flags(n - 2)["L1"]:
                        te.wait_ge(acts, ag1[n - 2])  # ps_g1[p2] WAR
                if f["L0"]:
                    te.wait_ge(xdma, 16 * (xd_cnt[n - 1] if n >= 1 else 1))
                    if n >= 2 and flags(n - 2)["L0"]:
                        te.wait_ge(acts, ag0[n - 2])  # ps_g0[p2] WAR
                mm_g0 = None
                for k in range(8):
                    if f["L1"]:
                        te.matmul(
                            ps_g1[p2][:, :],
                            Gb[(n - 2) % 3][:, k * 128:k * 128 + 64],
                            W1[:, k * 512:(k + 1) * 512],
                            start=(k == 0), stop=False,
                        )
                    if f["L0"] and k < 4:
                        te.matmul(
                            ps_g0[p2][:, :],
                            XT[:, p2 * 256 + k * 64:p2 * 256 + (k + 1) * 64],
                            W0[:, k * 512:(k + 1) * 512],
                            start=(k == 0), stop=False,
                        )
                    if f["L0"] and k == 4:
                        mm_g0 = te.matmul(
                            ps_g0[p2][:, :], ONES[:, :], W0[:, 12 * 512:13 * 512],
                            start=False, stop=(not f["L0dep"]),
                        )
                # keep-warm: the HAM clock gate only promotes to 2.4GHz
                # after ~3.4us of SUSTAINED PE busy, so fill the exchange
                # flight window with back-to-back dummy matmuls (unused
                # partition half of the gates1 PSUM bank).  A/C (~3us) plus
                # these (~2us) end just before the typical arrival, so the
                # D/E burst starts on a warm clock.
                if f["L0dep"] or f["Edep"]:
                    for w in range(4):
                        te.matmul(
                            ps_g1b[p2][0:64, :], ONES[:, :], W0[:, 0:512],
                            start=True, stop=True, skip_group_check=True,
                        )
                    for w in range(4):
                        te.wait_ge(
                            rs[(n - 1) % 3], max(1, rth(n - 1) - 14 + 4 * w)
                        )
                        te.matmul(
                            ps_g1b[p2][0:64, :], ONES[:, :], W0[:, 0:512],
                            start=True, stop=True, skip_group_check=True,
                        )
                # --- D+E pairs: both consume exchange n-1 ---
                if f["L0dep"] or f["Edep"]:
                    te.wait_ge(rs[(n - 1) % 3], rth(n - 1))
                for k in range(8):
                    if f["L0dep"]:
                        mm_g0 = te.matmul(
                            ps_g0[p2][:, :],
                            Gb[(n - 1) % 3][:, k * 128:k * 128 + 64],
                            W0[:, (4 + k) * 512:(5 + k) * 512],
                            start=False, stop=(k == 7),
                        )
                    if f["Edep"]:
                        te.matmul(
                            ps_g1[p2][:, :],
                            Gb[(n - 1) % 3][:, k * 128 + 64:(k + 1) * 128],
                            W1[:, (8 + k) * 512:(9 + k) * 512],
                            start=False, stop=False,
                        )
                if f["L0"]:
                    mm_g0.then_inc(pe, 1)
                if f["L1"]:
                    te.matmul(
                        ps_g1[p2][:, :], ONES[:, :], W1[:, 16 * 512:17 * 512],
                        start=False, stop=True,
                    ).then_inc(pe, 1)
                # --- T: one full transpose of this superstep's s pair ---
                if f["X"]:
                    if f["L1"]:
                        te.wait_ge(dve, ds1[n])
                    elif f["L0"]:
                        te.wait_ge(dve, ds0[n])
                    if n >= 2:
                        te.wait_ge(dve, dcp[n - 2])  # ps_t[p2] WAR
                    te.transpose(
                        ps_t[p2][:, :], sS[p2][:, :], IDN[:, :],
                    ).then_inc(pe, 1)
            # ---- epilogue GEMMs ----
            for j in range(TSH):
                te.wait_ge(edma, 16 * (j + 1))
                if j >= 2:
                    te.wait_ge(acts, ATOT + j * 2 - 2)  # ps_e WAR
                mm_e = None
                for k in range(8):
                    mm_e = te.matmul(
                        ps_e[j % 2][:, :],
                        es1[j % 2][:, k * 64:(k + 1) * 64],
                        P1S[:, k * 512:(k + 1) * 512],
                        start=(k == 0), stop=(k == 7),
                    )
                mm_e.then_inc(pe, 1)

        # ================= SCALAR (ACT) =================
        @block.scalar
        def _(sc):
            for n in range(NS):
                f = flags(n)
                p2 = n % 2
                # gate order [i, f, o, g]: one sigmoid + one tanh per layer
                if f["L0"]:
                    sc.wait_ge(pe, pe_g0[n])
                    sc.activation(actb[0:64, 0:384], ps_g0[p2][:, 0:384], AF.Sigmoid)
                    sc.activation(
                        actb[0:64, 384:512], ps_g0[p2][:, 384:512], AF.Tanh
                    ).then_inc(acts, 1)
                if f["L1"]:
                    sc.wait_ge(pe, pe_g1[n])
                    sc.activation(actb[64:128, 0:384], ps_g1[p2][:, 0:384], AF.Sigmoid)
                    sc.activation(
                        actb[64:128, 384:512], ps_g1[p2][:, 384:512], AF.Tanh
                    ).then_inc(acts, 1)
                if f["L0"]:
                    sc.wait_ge(dve, dc0[n])
                    sc.activation(
                        thc[0:64, 0:128], cbuf[0:64, 0:128], AF.Tanh
                    ).then_inc(acts, 1)
                if f["L1"]:
                    sc.wait_ge(dve, dc1[n])
                    sc.activation(
                        thc[64:128, 0:128], cbuf[64:128, 0:128], AF.Tanh
                    ).then_inc(acts, 1)
            for j in range(TSH):
                sc.wait_ge(dve, DTOT + j * 4 + 1)
                if j >= 1:
                    sc.wait_ge(ydma, 16 * j)  # ebuf WAR vs output DMA
                sc.activation(
                    emx[:, 1:2], emx[:, 0:1], AF.Copy, scale=-1.0
                ).then_inc(acts, 1)
                sc.wait_ge(acts, ATOT + j * 2 + 1)
                sc.activation(
                    ebuf[:, :], ps_e[j % 2][:, :], AF.Exp, bias=emx[:, 1:2]
                ).then_inc(acts, 1)

        # ================= VECTOR (DVE) =================
        @block.vector
        def _(ve):
            for n in range(NS):
                f = flags(n)
                p2 = n % 2
                # layout [i, f, o, g]: c = f*c + i*tanh(g); s = o*tanh(c)
                if f["L0"]:
                    ve.wait_ge(acts, ag0[n])
                    ve.tensor_mul(cbuf[0:64, :], actb[0:64, 128:256], cbuf[0:64, :])
                    ve.tensor_mul(actb[0:64, 0:128], actb[0:64, 0:128], actb[0:64, 384:512])
                    ve.tensor_add(
                        cbuf[0:64, :], cbuf[0:64, :], actb[0:64, 0:128]
                    ).then_inc(dve, 1)
                if f["L1"]:
                    ve.wait_ge(acts, ag1[n])
                    ve.tensor_mul(cbuf[64:128, :], actb[64:128, 128:256], cbuf[64:128, :])
                    ve.tensor_mul(actb[64:128, 0:128], actb[64:128, 0:128], actb[64:128, 384:512])
                    ve.tensor_add(
                        cbuf[64:128, :], cbuf[64:128, :], actb[64:128, 0:128]
                    ).then_inc(dve, 1)
                if f["L0"]:
                    ve.wait_ge(acts, at0[n])
                    ve.tensor_mul(
                        sS[p2][0:64, :], actb[0:64, 256:384], thc[0:64, :]
                    ).then_inc(dve, 1)
                if f["L1"]:
                    ve.wait_ge(acts, at1[n])
                    ve.tensor_mul(
                        sS[p2][64:128, :], actb[64:128, 256:384], thc[64:128, :]
                    ).then_inc(dve, 1)
                if f["X"]:
                    ve.wait_ge(pe, pe_t[n])
                    if n >= 2:
                        ve.wait_ge(ls, 16 * cnt_x(n - 2))  # SS[p2] WAR
                    ve.tensor_copy(SS[p2][:, :], ps_t[p2][:, :]).then_inc(dve, 1)
            dbase = DTOT
            for j in range(TSH):
                ve.wait_ge(pe, PTOT + j + 1)
                if j >= 1:
                    ve.wait_ge(acts, ATOT + j * 2 - 1)
                ve.tensor_reduce(
                    emx[:, 0:1], ps_e[j % 2][:, :],
                    mybir.AxisListType.X, mybir.AluOpType.max,
                ).then_inc(dve, 1)
                ve.wait_ge(acts, ATOT + j * 2 + 2)
                ve.tensor_reduce(
                    emx[:, 4:5], ebuf[:, :],
                    mybir.AxisListType.X, mybir.AluOpType.add,
                ).then_inc(dve, 1)
                ve.wait_ge(dve, dbase + j * 4 + 2)
                ve.reciprocal(emx[:, 2:3], emx[:, 4:5]).then_inc(dve, 1)
                ve.wait_ge(dve, dbase + j * 4 + 3)
                ve.tensor_scalar_mul(
                    ebuf[:, :], ebuf[:, :], emx[:, 2:3]
                ).then_inc(dve, 1)

    nc.compile()
    return nc


def _prep_inputs(inputs, t_steps=T):
    bf = ml_dtypes.bfloat16
    images = np.asarray(inputs["images"], np.float32)
    captions = np.asarray(inputs["captions"])
    table = np.asarray(inputs["embed_table"], np.float32)
    W_ih = np.asarray(inputs["W_ih"], np.float32)
    W_hh = np.asarray(inputs["W_hh"], np.float32)
    W_hr = np.asarray(inputs["W_hr"], np.float32)
    bsum = (np.asarray(inputs["b_ih"], np.float32)
            + np.asarray(inputs["b_hh"], np.float32))

    P0, P1 = W_hr[0], W_hr[1]
    M00 = W_hh[0] @ P0
    M10 = W_ih[1] @ P0
    M11 = W_hh[1] @ P1

    emb = table[captions[:, :-1]]
    X = np.concatenate([images, emb], axis=1)  # [B, T, E]
    xT = np.ascontiguousarray(
        X.transpose(2, 1, 0)[:, :t_steps, :].reshape(E, t_steps * B)
    ).astype(bf)

    ones = np.zeros((128, 64), bf)
    ones[0, :] = 1
    iden = np.eye(128, dtype=np.float32).astype(bf)
    p1w = np.ascontiguousarray(P1.T.reshape(8, 128, 512)).astype(bf)

    in_maps = []
    for r in range(NCORES):
        # gate row order [i, f, o, g] so one sigmoid covers cols 0:384
        rows = np.concatenate(
            [np.arange(g * 1024 + r * 128, g * 1024 + (r + 1) * 128)
             for g in (0, 1, 3, 2)]
        )
        w0 = np.zeros((13, 128, 512), bf)
        w0[0:4] = W_ih[0][rows].T.reshape(4, 128, 512).astype(bf)
        w0[4:12] = M00[rows].T.reshape(8, 128, 512).astype(bf)
        bt = np.zeros((128, 512), np.float32)
        bt[0, :] = bsum[0][rows]
        w0[12] = bt.astype(bf)
        w1 = np.zeros((17, 128, 512), bf)
        w1[0:8] = M10[rows].T.reshape(8, 128, 512).astype(bf)
        w1[8:16] = M11[rows].T.reshape(8, 128, 512).astype(bf)
        bt1 = np.zeros((128, 512), np.float32)
        bt1[0, :] = bsum[1][rows]
        w1[16] = bt1.astype(bf)
        in_maps.append({
            "w0": w0, "w1": w1, "p1w": p1w, "xT": xT,
            "ones": ones, "iden": iden,
            "rank": np.array([[r]], np.int32),
            "rank16": np.array([[r * (t_steps // NCORES)]], np.int32),
        })
    return in_maps


def kernel(**inputs):
    global LAST_EXEC_NS
    if TRACE:
        _install_trace_hook()
    if "nc" not in _CACHE:
        _CACHE["nc"] = build(T)
    nc = _CACHE["nc"]
    in_maps = _prep_inputs(inputs)
    res = run_bass_kernel_spmd(
        nc, in_maps, core_ids=list(range(8)), trace=TRACE
    )
    LAST_EXEC_NS = res.exec_time_ns
    out = np.concatenate([res.results[r]["y"] for r in range(8)], axis=1)
    return out.astype(np.float32)


if __name__ == "__main__":
    pass


def debug_run(inputs, t_steps=8):
    if TRACE:
        _install_trace_hook()
    nc = build(t_steps, dump=True)
    in_maps = _prep_inputs(inputs, t_steps)
    res = run_bass_kernel_spmd(nc, in_maps, core_ids=list(range(8)), trace=TRACE)
    y = np.concatenate([res.results[r]["y"] for r in range(8)], axis=1)
    s1d = [res.results[r]["s1store"] for r in range(8)]
    _CACHE["dbg"] = [res.results[r].get("dbg") for r in range(8)]
    return y.astype(np.float32), s1d, res.exec_time_ns
